# revision 10
# baseline (speedup 1.0000x reference)
"""Trainium2 Bass kernel for HQNN-Quanv (B=1024, 1x28x28, K=2).

Math: circuit weights are zero, so RX/RY are identity and the circuit is
three CNOTs. With c_k = cos(pi p_k) per 2x2 patch: <Z0>=c0, <Z1>=c1,
<Z2>=c0c2, <Z3>=c0c2c3, then the 10-way dense layer. The device computes
s = sin(pi(x-0.5)) = -cos(pi x); sign flips and the <Z1> re-indexing fold
into host-prepared weights. Pure data parallel: batch/8 per core.

The linear term needs only 6 slot-chunks: row-27 lin weights are
structurally zero (W0/W1 cover i<27), so the dense bias rides in a
zero-weight slot (756) of chunk 5 instead of a 7th chunk.

Tuned against the measured latency model (HWDGE dispatch ~0.65us, DGE
queue delay ~0.78us, DMA sem prop 900ns; a DMA's data is consumable at
its own dispatch-end + 780 + transfer + 900, so serialized dispatch
slots gate the pipeline; ACT ~1.05ns/col + ~130ns/instr; PE fp16 matmul
107ns/128 cols at mid p-state, 56ns warmed):
  - 3 input DMAs [b][w|lin][c]: the b-copy's sem gates the end-chain so
    b rides dispatch slot 1; merging w+lin into one slot pulls c's
    doorbell ~600ns earlier, making the ACT sine stream (b -> lin -> c)
    completely gap-free from the b-sem to stream end.
  - 5 SIN activations (b whole, lin 384/384, c in halves) so the
    e2 = sl*sb and e3 = e2*sc DVE products (in halves) and the final
    matmuls chase the ACT stream.
  - 18 accumulating fp16 matmuls into one PSUM tile; 26 dummy matmuls
    on memset scratch warm the PE p-state (1.2 -> 2.4 GHz) first.
  - DVE copies PSUM -> SBUF, padded to 16 partitions (a 10-partition
    DMA dispatch measured ~270ns slower); single output DMA.
  - Fixed-overhead surgery: semaphore cleanup moved to program start as
    RANGE_CLEARs at the head of the Pool stream (re-run safe, verified
    by double execution), tail reduced to one sync drain with
    final-value waits, const-AP memsets dropped (Sin bias is a
    tile-tracked zero tensor), the 5-engine initial barrier reduced to
    Pool -> SP ordering, unused Activation HWDGE queue dropped,
    walrus 1-wait limit handled by NoOp splits.
"""

import sys

if "/opt/trn_rl_repo" not in sys.path:
    sys.path.insert(0, "/opt/trn_rl_repo")

import numpy as np

B = 1024
NCORES = 8
BC = B // NCORES  # 128 images per core
H = 28
F = 27
NLIN = 6  # lin chunks; row-27 weights are zero so slots 756+ fold away
NE = 6  # ceil(756/128) chunks for the E2/E3 terms
FREE_LIN = NLIN * 128  # 768
FREE_E = NE * 128  # 768
WCOLS = (NLIN + 2 * NE) * 10  # 180
BIAS_SLOT = 756  # zero-weight slot (row 27) in chunk 5
NCOLS = WCOLS + FREE_LIN + 2 * FREE_E  # 2484: [w | lin | b | c]
NWARM = 26  # dummy matmuls for PE p-state ramp (drain before real mms)

_cached_nc = None


def _slim_drain_and_barrier(self, tick_clock, wait_clock):
    """One-shot tail: final-value waits on a single sync drain; semaphore
    cleanup happens at program start instead."""
    from concourse.vector_clock import ScopedClock

    drain_inst = self.nc.sync.drain()
    wait_clock.add_sem_waits(
        drain_inst.ins, ScopedClock({None: tick_clock.global_clock})
    )
    popped = self.nc._tile_sem_poison_stack.pop()
    assert popped is self._sem_poison


def build_nc():
    import concourse.bass as bass
    import concourse.tile as tile
    import concourse.mybir as mybir
    from concourse.bass import _add_dep_helper, compact_to_ranges

    nc = bass.Bass("TRN2", target_bir_lowering=False, debug=False)
    f16 = mybir.dt.float16
    f32 = mybir.dt.float32

    xd = nc.dram_tensor("xd", [128, NCOLS], f16, kind="ExternalInput")
    y = nc.dram_tensor("y", [16, BC], f32, kind="ExternalOutput")

    # column boundaries in xd / xt: [b | w | lin | c]
    Bc0, Bc1 = 0, FREE_E  # b-copy first (its sem gates the end-chain)
    W0_, W1_ = Bc1, Bc1 + WCOLS  # weights
    L0 = W1_  # lin start
    L2 = L0 + FREE_LIN  # lin end
    C0 = L2  # c start
    C1 = L2 + FREE_E  # c end

    tc = tile.TileContext(nc)
    tc._drain_and_barrier = _slim_drain_and_barrier.__get__(tc)
    with tc:
        with (
            tc.tile_pool(name="p", bufs=1) as pool,
            tc.tile_pool(name="ps", bufs=1, space="PSUM") as pp,
        ):
            xt = pool.tile([128, NCOLS], f16)
            nc.sync.dma_start(xt[:, Bc0:Bc1], xd.ap()[:, Bc0:Bc1])
            nc.sync.dma_start(xt[:, W0_:L2], xd.ap()[:, W0_:L2])
            nc.sync.dma_start(xt[:, C0:C1], xd.ap()[:, C0:C1])
            wt = xt[:, W0_:W1_]

            # PE p-state warmup: matmuls on never-written scratch (values
            # irrelevant, no waits) keep the PE continuously busy so the
            # real accumulation chain runs at the ramped clock.
            warm_src = pool.tile([128, 138], f16)
            nc.gpsimd.memset(warm_src[:], 0.0)
            ys = pool.tile([16, BC], f32)
            nc.gpsimd.memset(ys[:], 0.0)
            bias_t = pool.tile([128, 1], f32)
            nc.gpsimd.memset(bias_t[:], 0.0)
            warm_ps = pp.tile([10, 128], f32)
            for _ in range(NWARM):
                nc.tensor.matmul(
                    warm_ps[:], warm_src[:, 0:10], warm_src[:, 10:138],
                    start=True, stop=True,
                )

            st = pool.tile([128, FREE_LIN + 2 * FREE_E], f16)
            sl = st[:, 0:FREE_LIN]
            sb = st[:, FREE_LIN : FREE_LIN + FREE_E]
            sc = st[:, FREE_LIN + FREE_E : FREE_LIN + 2 * FREE_E]

            sin = mybir.ActivationFunctionType.Sin
            pi = float(np.pi)
            hE = FREE_E // 2  # 384
            acts = []
            acts.append(
                nc.scalar.activation(sb[:], xt[:, Bc0:Bc1], sin, bias=bias_t[:, 0:1], scale=pi)
            )
            acts.append(
                nc.scalar.activation(st[:, 0:384], xt[:, L0 : L0 + 384], sin, bias=bias_t[:, 0:1], scale=pi)
            )
            acts.append(
                nc.scalar.activation(
                    st[:, 384:FREE_LIN], xt[:, L0 + 384 : L2], sin, bias=bias_t[:, 0:1], scale=pi
                )
            )
            acts.append(
                nc.scalar.activation(
                    sc[:, 0:hE], xt[:, C0 : C0 + hE], sin, bias=bias_t[:, 0:1], scale=pi
                )
            )
            acts.append(
                nc.scalar.activation(
                    sc[:, hE:FREE_E], xt[:, C0 + hE : C1], sin, bias=bias_t[:, 0:1], scale=pi
                )
            )
            for prev, nxt in zip(acts, acts[1:]):
                _add_dep_helper(nxt.ins, prev.ins, False, "pin ACT order")

            e2 = pool.tile([128, FREE_E], f16)
            nc.vector.tensor_mul(e2[:, 0:hE], sl[:, 0:hE], sb[:, 0:hE])
            nc.vector.tensor_mul(e2[:, hE:FREE_E], sl[:, hE:FREE_E], sb[:, hE:FREE_E])
            e3 = pool.tile([128, FREE_E], f16)
            nc.vector.tensor_mul(e3[:, 0:hE], e2[:, 0:hE], sc[:, 0:hE])
            nc.vector.tensor_mul(e3[:, hE:FREE_E], e2[:, hE:FREE_E], sc[:, hE:FREE_E])

            yp = pp.tile([10, BC], f32)
            nmm = NLIN + 2 * NE
            i = 0
            for t in range(NLIN):
                nc.tensor.matmul(
                    yp[:],
                    wt[:, t * 10 : (t + 1) * 10],
                    sl[:, t * 128 : (t + 1) * 128],
                    start=(i == 0),
                    stop=(i == nmm - 1),
                )
                i += 1
            for src, wofs in ((e2, NLIN * 10), (e3, (NLIN + NE) * 10)):
                for t in range(NE):
                    nc.tensor.matmul(
                        yp[:],
                        wt[:, wofs + t * 10 : wofs + (t + 1) * 10],
                        src[:, t * 128 : (t + 1) * 128],
                        start=(i == 0),
                        stop=(i == nmm - 1),
                    )
                    i += 1

            nc.vector.tensor_copy(ys[0:10, :], yp[:])
            nc.sync.dma_start(y.ap(), ys[:])

    # Narrow front-clear: only the sems actually allocated (minus live
    # barrier sems), emitted post-tile then hoisted before the barrier.
    free = set(nc.free_semaphores)
    used = [
        s
        for s in nc._kernel_sem_range
        if s not in free and s not in nc.barrier_sems
    ]
    front_insts = [nc.gpsimd.sem_clear(r).ins for r in compact_to_ranges(used)]

    _slim_main_block(nc, front_insts)
    _split_multi_waits(nc)
    nc.m.queues = [q for q in nc.m.queues if q.name != "qActDynamicHW"]
    return nc


def _slim_main_block(nc, front_insts):
    """Preamble surgery on block `main`:
      - drop the 3 unused const-AP memsets (keep const-float32-0.0, the
        Sin bias operand);
      - hoist the front-clear (Pool) before the const memset;
      - replace the 5-engine initial barrier with a Pool release-update
        plus waits on SP and Activation only. Ordering needed: Pool's
        sem-clear + const memset must precede SP's first DMA sem update
        and ACT's const read. PE/DVE are transitively gated through tile
        sems (warm_src memset, ACT outputs). The release sem is waited
        with >= so the +2 leftover per run is re-run safe.
    front_insts were emitted into the trailing block; move them."""
    import concourse.mybir as mybir

    blocks = nc.m.functions[0].blocks
    front_set = {id(i) for i in front_insts}
    for blk in blocks:
        blk.instructions[:] = [
            i for i in blk.instructions if id(i) not in front_set
        ]
    blk = blocks[0]
    assert blk.name == "main"

    keep_insts = []
    eng = mybir.EngineType
    release_inst = None
    n_memset = 0
    for inst in blk.instructions:
        tn = type(inst).__name__
        nm = inst.name
        if tn == "InstMemset":
            n_memset += 1
            continue  # all const-APs unused (Sin bias is a tile now)
        if tn == "InstDrain" and not nm.startswith("I-splitw"):
            continue  # preamble barrier drains
        if tn == "InstEventSemaphore" and nm.startswith("barrier_"):
            si = inst.sync_info
            if inst.engine == eng.Pool:
                if si is not None and si.on_wait:
                    continue  # gather-wait
                release_inst = inst  # release update: hoist to block head
                continue
            elif inst.engine in (eng.PE, eng.DVE, eng.Activation):
                continue
            # SP's release-wait: keep (orders first DMA after front-clear)
        keep_insts.append(inst)
    assert n_memset == 4, n_memset
    assert release_inst is not None
    # Pool runs [clears, release] as its very first instructions (RANGE_CLEAR
    # is a pure sem-file op, independent of the base-register preamble), so
    # SP's release-wait clears as early as possible.
    blk.instructions[:] = front_insts + [release_inst] + keep_insts


def _split_multi_waits(nc):
    """Walrus allows only one sync-wait per instruction; split any
    multi-wait instruction into preceding single-wait NoOps."""
    import concourse.mybir as mybir

    ctr = 0
    for blk in nc.m.functions[0].blocks:
        new_insts = []
        changed = False
        for inst in blk.instructions:
            si = inst.sync_info
            if si is not None and si.on_wait and len(si.on_wait) > 1:
                waits = list(si.on_wait)
                for w in waits[:-1]:
                    nop = mybir.InstNoOp(name=f"I-splitw-{ctr}", ins=[], outs=[])
                    ctr += 1
                    nop.engine = inst.engine
                    nop.sync_info = mybir.SyncInfo(on_wait=[w], on_update=[])
                    nc.register_instruction(nop, overwrite=True)
                    new_insts.append(nop)
                si.on_wait = waits[-1:]
                changed = True
            new_insts.append(inst)
        if changed:
            blk.instructions[:] = new_insts


def prep_x_core(xs):
    """xs: (BC, 28, 28) float32 -> (128, 2432) fp16 [lin | b | c]."""
    u2 = (xs.reshape(BC, H * H) - 0.5).astype(np.float16)
    ut = u2.T  # (784, BC)

    ulin = np.zeros((FREE_LIN, BC), np.float16)
    ulin[:756] = ut[:756]  # slots 756-783 (row 27) carry zero lin weight
    ulin[BIAS_SLOT] = 0.5  # bias slot: sin(pi*0.5) = 1
    xlin = ulin.reshape(NLIN, 128, BC).transpose(1, 0, 2).reshape(128, FREE_LIN)

    ub = np.zeros((FREE_E, BC), np.float16)
    ub[:756] = ut[28:784]
    xbm = ub.reshape(NE, 128, BC).transpose(1, 0, 2).reshape(128, FREE_E)

    uc = np.zeros((FREE_E, BC), np.float16)
    uc[:755] = ut[29:784]
    phi = np.arange(FREE_E)
    uc[phi % 28 == 27] = 0.0  # j==27 slots are weight-masked; keep finite
    xcm = uc.reshape(NE, 128, BC).transpose(1, 0, 2).reshape(128, FREE_E)

    return xlin, xbm, xcm


def prep_w(W, b):
    """W: (10, 2916), b: (10,) -> (128, WCOLS) fp16. Sign folds for
    s = -cos(pi x): lin -> -A, E2 -> +W2, E3 -> -W3."""
    W = W.astype(np.float32)
    W0 = W[:, 0:729].reshape(10, F, F)
    W1 = W[:, 729:1458].reshape(10, F, F)
    W2 = W[:, 1458:2187].reshape(10, F, F)
    W3 = W[:, 2187:2916].reshape(10, F, F)

    A = np.zeros((10, H, H), np.float32)
    A[:, :F, :F] += W0
    A[:, :F, 1:H] += W1

    wlin = np.zeros((10, FREE_LIN), np.float32)
    wlin[:, :756] = -A.reshape(10, H * H)[:, :756]
    wlin[:, BIAS_SLOT] = b
    wlin_p = wlin.reshape(10, NLIN, 128).transpose(2, 1, 0).reshape(128, NLIN * 10)

    w2s = np.zeros((10, FREE_E), np.float32)
    w2s[:, :756].reshape(10, F, H)[:, :, :F] = W2
    w2_p = w2s.reshape(10, NE, 128).transpose(2, 1, 0).reshape(128, NE * 10)

    w3s = np.zeros((10, FREE_E), np.float32)
    w3s[:, :756].reshape(10, F, H)[:, :, :F] = -W3
    w3_p = w3s.reshape(10, NE, 128).transpose(2, 1, 0).reshape(128, NE * 10)

    return np.concatenate([wlin_p, w2_p, w3_p], axis=1).astype(np.float16)


def _get_nc():
    global _cached_nc
    if _cached_nc is None:
        _cached_nc = build_nc()
    return _cached_nc


def _make_in_maps(inputs):
    x = np.asarray(inputs["x"], np.float32)
    W = np.asarray(inputs["W"], np.float32)
    b = np.asarray(inputs["b"], np.float32)
    wd = prep_w(W, b)
    in_maps = []
    for k in range(NCORES):
        xs = x[k * BC : (k + 1) * BC, 0]
        xl, xb, xc = prep_x_core(xs)
        xp = np.concatenate([xb, wd, xl, xc], axis=1)
        in_maps.append({"xd": xp})
    return in_maps


def run(inputs, trace=False, **kwargs):
    from concourse.bass_utils import run_bass_kernel_spmd

    nc = _get_nc()
    in_maps = _make_in_maps(inputs)
    res = run_bass_kernel_spmd(
        nc, in_maps, core_ids=list(range(NCORES)), trace=trace, **kwargs
    )
    out = np.concatenate([r["y"][:10].T for r in res.results], axis=0)
    return out, res


def kernel(**inputs) -> np.ndarray:
    out, _ = run(inputs, trace=False)
    return out


# revision 12
# speedup vs baseline: 1.0954x; 1.0954x over previous
"""Trainium2 Bass kernel for HQNN-Quanv (B=1024, 1x28x28, K=2).

Math: circuit weights are zero, so RX/RY are identity and the circuit is
three CNOTs. With c_k = cos(pi p_k) per 2x2 patch: <Z0>=c0, <Z1>=c1,
<Z2>=c0c2, <Z3>=c0c2c3, then the 10-way dense layer. The device computes
s = sin(pi(x-0.5)) = -cos(pi x); sign flips and the <Z1> re-indexing fold
into host-prepared weights. Pure data parallel: batch/8 per core. The
linear term needs only 6 slot-chunks (row-27 lin weights are zero), with
the dense bias in a zero-weight slot (756) of chunk 5.

Schedule, tuned against the measured latency model (a DMA's data is
consumable at its own dispatch-end + 780 + transfer + 900ns sem-prop;
ACT ~1.2ns/col; PE fp16 matmul 56ns/128 cols when p-state-warmed):
  - 3 input DMAs [b][w|lin][c]: the b-copy's sem gates the end-chain so
    b rides dispatch slot 1; merging w+lin pulls c's doorbell earlier,
    making the sine stream (b -> lin 384/384 -> c 384/384) gap-free.
  - The Sin ACT table load (1.3us + 1.4us hidden drain) is pre-placed in
    the main block (InstLoadActFuncSet, set 9 trig_and_small) so it
    completes during the walrus-init tail instead of gating the first
    sine; Bacc.insert_act_table_loads sees it and skips its own.
  - e2 = sl*sb and e3 = e2*sc in halves on DVE chase the ACT stream;
    18 accumulating fp16 matmuls into one PSUM tile; 26 dummy matmuls
    warm the PE p-state first.
  - DVE copies PSUM -> SBUF padded to 16 partitions (10-partition DMA
    dispatch is ~270ns slower); single output DMA.
  - Fixed-overhead surgery: sem cleanup as RANGE_CLEARs at the head of
    the Pool stream (re-run safe), one-drain tail with final-value
    waits, no const-AP memsets (Sin bias is a tile-tracked zero
    tensor), initial barrier reduced to Pool -> SP, unused Activation
    HWDGE queue dropped, walrus 1-wait limit handled by NoOp splits.
"""

import sys

if "/opt/trn_rl_repo" not in sys.path:
    sys.path.insert(0, "/opt/trn_rl_repo")

import numpy as np

B = 1024
NCORES = 8
BC = B // NCORES  # 128 images per core
H = 28
F = 27
NLIN = 6  # lin chunks; row-27 weights are zero so slots 756+ fold away
NE = 6  # ceil(756/128) chunks for the E2/E3 terms
FREE_LIN = NLIN * 128  # 768
FREE_E = NE * 128  # 768
WCOLS = (NLIN + 2 * NE) * 10  # 180
BIAS_SLOT = 756  # zero-weight slot (row 27) in chunk 5
NCOLS = WCOLS + FREE_LIN + 2 * FREE_E  # 2484: [w | lin | b | c]
NWARM = 26  # dummy matmuls for PE p-state ramp (drain before real mms)

_cached_nc = None


def _slim_drain_and_barrier(self, tick_clock, wait_clock):
    """One-shot tail: final-value waits on a single sync drain; semaphore
    cleanup happens at program start instead."""
    from concourse.vector_clock import ScopedClock

    drain_inst = self.nc.sync.drain()
    wait_clock.add_sem_waits(
        drain_inst.ins, ScopedClock({None: tick_clock.global_clock})
    )
    popped = self.nc._tile_sem_poison_stack.pop()
    assert popped is self._sem_poison


def build_nc():
    import concourse.bass as bass
    import concourse.tile as tile
    import concourse.mybir as mybir
    from concourse.bass import _add_dep_helper, compact_to_ranges

    nc = bass.Bass("TRN2", target_bir_lowering=False, debug=False)
    f16 = mybir.dt.float16
    f32 = mybir.dt.float32

    xd = nc.dram_tensor("xd", [128, NCOLS], f16, kind="ExternalInput")
    y = nc.dram_tensor("y", [16, BC], f32, kind="ExternalOutput")

    # column boundaries in xd / xt: [b | w | lin | c]
    Bc0, Bc1 = 0, FREE_E  # b-copy first (its sem gates the end-chain)
    W0_, W1_ = Bc1, Bc1 + WCOLS  # weights
    L0 = W1_  # lin start
    L2 = L0 + FREE_LIN  # lin end
    C0 = L2  # c start
    C1 = L2 + FREE_E  # c end

    tc = tile.TileContext(nc)
    tc._drain_and_barrier = _slim_drain_and_barrier.__get__(tc)
    with tc:
        with (
            tc.tile_pool(name="p", bufs=1) as pool,
            tc.tile_pool(name="ps", bufs=1, space="PSUM") as pp,
        ):
            xt = pool.tile([128, NCOLS], f16)
            nc.sync.dma_start(xt[:, Bc0:Bc1], xd.ap()[:, Bc0:Bc1])
            nc.sync.dma_start(xt[:, W0_:L2], xd.ap()[:, W0_:L2])
            nc.sync.dma_start(xt[:, C0:C1], xd.ap()[:, C0:C1])
            wt = xt[:, W0_:W1_]

            # PE p-state warmup: matmuls on never-written scratch (values
            # irrelevant, no waits) keep the PE continuously busy so the
            # real accumulation chain runs at the ramped clock.
            warm_src = pool.tile([128, 138], f16)
            nc.gpsimd.memset(warm_src[:], 0.0)
            ys = pool.tile([16, BC], f32)
            nc.gpsimd.memset(ys[:], 0.0)
            bias_t = pool.tile([128, 1], f32)
            nc.gpsimd.memset(bias_t[:], 0.0)
            warm_ps = pp.tile([10, 128], f32)
            for _ in range(NWARM):
                nc.tensor.matmul(
                    warm_ps[:], warm_src[:, 0:10], warm_src[:, 10:138],
                    start=True, stop=True,
                )

            st = pool.tile([128, FREE_LIN + 2 * FREE_E], f16)
            sl = st[:, 0:FREE_LIN]
            sb = st[:, FREE_LIN : FREE_LIN + FREE_E]
            sc = st[:, FREE_LIN + FREE_E : FREE_LIN + 2 * FREE_E]

            sin = mybir.ActivationFunctionType.Sin
            pi = float(np.pi)
            hE = FREE_E // 2  # 384
            acts = []
            acts.append(
                nc.scalar.activation(sb[:], xt[:, Bc0:Bc1], sin, bias=bias_t[:, 0:1], scale=pi)
            )
            acts.append(
                nc.scalar.activation(st[:, 0:384], xt[:, L0 : L0 + 384], sin, bias=bias_t[:, 0:1], scale=pi)
            )
            acts.append(
                nc.scalar.activation(
                    st[:, 384:FREE_LIN], xt[:, L0 + 384 : L2], sin, bias=bias_t[:, 0:1], scale=pi
                )
            )
            acts.append(
                nc.scalar.activation(
                    sc[:, 0:hE], xt[:, C0 : C0 + hE], sin, bias=bias_t[:, 0:1], scale=pi
                )
            )
            acts.append(
                nc.scalar.activation(
                    sc[:, hE:FREE_E], xt[:, C0 + hE : C1], sin, bias=bias_t[:, 0:1], scale=pi
                )
            )
            for prev, nxt in zip(acts, acts[1:]):
                _add_dep_helper(nxt.ins, prev.ins, False, "pin ACT order")

            e2 = pool.tile([128, FREE_E], f16)
            nc.vector.tensor_mul(e2[:, 0:hE], sl[:, 0:hE], sb[:, 0:hE])
            nc.vector.tensor_mul(e2[:, hE:FREE_E], sl[:, hE:FREE_E], sb[:, hE:FREE_E])
            e3 = pool.tile([128, FREE_E], f16)
            nc.vector.tensor_mul(e3[:, 0:hE], e2[:, 0:hE], sc[:, 0:hE])
            nc.vector.tensor_mul(e3[:, hE:FREE_E], e2[:, hE:FREE_E], sc[:, hE:FREE_E])

            yp = pp.tile([10, BC], f32)
            nmm = NLIN + 2 * NE
            i = 0
            for t in range(NLIN):
                nc.tensor.matmul(
                    yp[:],
                    wt[:, t * 10 : (t + 1) * 10],
                    sl[:, t * 128 : (t + 1) * 128],
                    start=(i == 0),
                    stop=(i == nmm - 1),
                )
                i += 1
            for src, wofs in ((e2, NLIN * 10), (e3, (NLIN + NE) * 10)):
                for t in range(NE):
                    nc.tensor.matmul(
                        yp[:],
                        wt[:, wofs + t * 10 : wofs + (t + 1) * 10],
                        src[:, t * 128 : (t + 1) * 128],
                        start=(i == 0),
                        stop=(i == nmm - 1),
                    )
                    i += 1

            nc.vector.tensor_copy(ys[0:10, :], yp[:])
            nc.sync.dma_start(y.ap(), ys[:])

    # Narrow front-clear: only the sems actually allocated (minus live
    # barrier sems), emitted post-tile then hoisted before the barrier.
    free = set(nc.free_semaphores)
    used = [
        s
        for s in nc._kernel_sem_range
        if s not in free and s not in nc.barrier_sems
    ]
    front_insts = [nc.gpsimd.sem_clear(r).ins for r in compact_to_ranges(used)]

    # Pre-place the Sin ACT table load (set 9, trig_and_small) in the main
    # block so the ~1.3us load + ~1.4us drain run during the walrus-init
    # tail instead of gating the first sine: Bacc.insert_act_table_loads
    # sees Sin loaded on all paths and skips its own insertion.
    tbl = mybir.InstLoadActFuncSet(
        name=nc.get_next_instruction_name(), ins=[], outs=[], act_func_set_id=9
    )
    tbl.engine = mybir.EngineType.Activation
    nc.register_instruction(tbl)
    blk0 = nc.m.functions[0].blocks[0]
    bidx = next(
        i
        for i, inst in enumerate(blk0.instructions)
        if type(inst).__name__ == "InstUnconditionalBranch"
    )
    blk0.instructions.insert(bidx, tbl)

    _slim_main_block(nc, front_insts)
    _split_multi_waits(nc)
    nc.m.queues = [q for q in nc.m.queues if q.name != "qActDynamicHW"]
    return nc


def _slim_main_block(nc, front_insts):
    """Preamble surgery on block `main`:
      - drop the 3 unused const-AP memsets (keep const-float32-0.0, the
        Sin bias operand);
      - hoist the front-clear (Pool) before the const memset;
      - replace the 5-engine initial barrier with a Pool release-update
        plus waits on SP and Activation only. Ordering needed: Pool's
        sem-clear + const memset must precede SP's first DMA sem update
        and ACT's const read. PE/DVE are transitively gated through tile
        sems (warm_src memset, ACT outputs). The release sem is waited
        with >= so the +2 leftover per run is re-run safe.
    front_insts were emitted into the trailing block; move them."""
    import concourse.mybir as mybir

    blocks = nc.m.functions[0].blocks
    front_set = {id(i) for i in front_insts}
    for blk in blocks:
        blk.instructions[:] = [
            i for i in blk.instructions if id(i) not in front_set
        ]
    blk = blocks[0]
    assert blk.name == "main"

    keep_insts = []
    eng = mybir.EngineType
    release_inst = None
    n_memset = 0
    for inst in blk.instructions:
        tn = type(inst).__name__
        nm = inst.name
        if tn == "InstMemset":
            n_memset += 1
            continue  # all const-APs unused (Sin bias is a tile now)
        if tn == "InstDrain" and not nm.startswith("I-splitw"):
            continue  # preamble barrier drains
        if tn == "InstEventSemaphore" and nm.startswith("barrier_"):
            si = inst.sync_info
            if inst.engine == eng.Pool:
                if si is not None and si.on_wait:
                    continue  # gather-wait
                release_inst = inst  # release update: hoist to block head
                continue
            elif inst.engine in (eng.PE, eng.DVE, eng.Activation):
                continue
            # SP's release-wait: keep (orders first DMA after front-clear)
        keep_insts.append(inst)
    assert n_memset == 4, n_memset
    assert release_inst is not None
    # Pool runs [clears, release] as its very first instructions (RANGE_CLEAR
    # is a pure sem-file op, independent of the base-register preamble), so
    # SP's release-wait clears as early as possible.
    blk.instructions[:] = front_insts + [release_inst] + keep_insts


def _split_multi_waits(nc):
    """Walrus allows only one sync-wait per instruction; split any
    multi-wait instruction into preceding single-wait NoOps."""
    import concourse.mybir as mybir

    ctr = 0
    for blk in nc.m.functions[0].blocks:
        new_insts = []
        changed = False
        for inst in blk.instructions:
            si = inst.sync_info
            if si is not None and si.on_wait and len(si.on_wait) > 1:
                waits = list(si.on_wait)
                for w in waits[:-1]:
                    nop = mybir.InstNoOp(name=f"I-splitw-{ctr}", ins=[], outs=[])
                    ctr += 1
                    nop.engine = inst.engine
                    nop.sync_info = mybir.SyncInfo(on_wait=[w], on_update=[])
                    nc.register_instruction(nop, overwrite=True)
                    new_insts.append(nop)
                si.on_wait = waits[-1:]
                changed = True
            new_insts.append(inst)
        if changed:
            blk.instructions[:] = new_insts


def prep_x_core(xs):
    """xs: (BC, 28, 28) float32 -> (128, 2432) fp16 [lin | b | c]."""
    u2 = (xs.reshape(BC, H * H) - 0.5).astype(np.float16)
    ut = u2.T  # (784, BC)

    ulin = np.zeros((FREE_LIN, BC), np.float16)
    ulin[:756] = ut[:756]  # slots 756-783 (row 27) carry zero lin weight
    ulin[BIAS_SLOT] = 0.5  # bias slot: sin(pi*0.5) = 1
    xlin = ulin.reshape(NLIN, 128, BC).transpose(1, 0, 2).reshape(128, FREE_LIN)

    ub = np.zeros((FREE_E, BC), np.float16)
    ub[:756] = ut[28:784]
    xbm = ub.reshape(NE, 128, BC).transpose(1, 0, 2).reshape(128, FREE_E)

    uc = np.zeros((FREE_E, BC), np.float16)
    uc[:755] = ut[29:784]
    phi = np.arange(FREE_E)
    uc[phi % 28 == 27] = 0.0  # j==27 slots are weight-masked; keep finite
    xcm = uc.reshape(NE, 128, BC).transpose(1, 0, 2).reshape(128, FREE_E)

    return xlin, xbm, xcm


def prep_w(W, b):
    """W: (10, 2916), b: (10,) -> (128, WCOLS) fp16. Sign folds for
    s = -cos(pi x): lin -> -A, E2 -> +W2, E3 -> -W3."""
    W = W.astype(np.float32)
    W0 = W[:, 0:729].reshape(10, F, F)
    W1 = W[:, 729:1458].reshape(10, F, F)
    W2 = W[:, 1458:2187].reshape(10, F, F)
    W3 = W[:, 2187:2916].reshape(10, F, F)

    A = np.zeros((10, H, H), np.float32)
    A[:, :F, :F] += W0
    A[:, :F, 1:H] += W1

    wlin = np.zeros((10, FREE_LIN), np.float32)
    wlin[:, :756] = -A.reshape(10, H * H)[:, :756]
    wlin[:, BIAS_SLOT] = b
    wlin_p = wlin.reshape(10, NLIN, 128).transpose(2, 1, 0).reshape(128, NLIN * 10)

    w2s = np.zeros((10, FREE_E), np.float32)
    w2s[:, :756].reshape(10, F, H)[:, :, :F] = W2
    w2_p = w2s.reshape(10, NE, 128).transpose(2, 1, 0).reshape(128, NE * 10)

    w3s = np.zeros((10, FREE_E), np.float32)
    w3s[:, :756].reshape(10, F, H)[:, :, :F] = -W3
    w3_p = w3s.reshape(10, NE, 128).transpose(2, 1, 0).reshape(128, NE * 10)

    return np.concatenate([wlin_p, w2_p, w3_p], axis=1).astype(np.float16)


def _get_nc():
    global _cached_nc
    if _cached_nc is None:
        _cached_nc = build_nc()
    return _cached_nc


def _make_in_maps(inputs):
    x = np.asarray(inputs["x"], np.float32)
    W = np.asarray(inputs["W"], np.float32)
    b = np.asarray(inputs["b"], np.float32)
    wd = prep_w(W, b)
    in_maps = []
    for k in range(NCORES):
        xs = x[k * BC : (k + 1) * BC, 0]
        xl, xb, xc = prep_x_core(xs)
        xp = np.concatenate([xb, wd, xl, xc], axis=1)
        in_maps.append({"xd": xp})
    return in_maps


def run(inputs, trace=False, **kwargs):
    from concourse.bass_utils import run_bass_kernel_spmd

    nc = _get_nc()
    in_maps = _make_in_maps(inputs)
    res = run_bass_kernel_spmd(
        nc, in_maps, core_ids=list(range(NCORES)), trace=trace, **kwargs
    )
    out = np.concatenate([r["y"][:10].T for r in res.results], axis=0)
    return out, res


def kernel(**inputs) -> np.ndarray:
    out, _ = run(inputs, trace=False)
    return out
